# revision 1
# baseline (speedup 1.0000x reference)
"""Trainium2 Bass kernel for nn_DynamicSparseAttention.

Model (per batch b): QKV proj -> per-head scores [S,S] -> per-row exact 0.9
quantile threshold -> sparse mask (top ~205 of 2048 per row) -> softmax ->
att @ V -> block-diag distill (16x16) -> sigmoid gate mix -> out proj.

Sharding: 8 cores = (batch b in 0..3) x (head-pair hp in 0..1); each core
computes 2 heads of one batch and a partial out-projection [256, 2048]^T.
Host sums the two partials per batch.

Per-core algorithm per head (the per-row quantile is the crux):
 1. scores^T-free layout-1 matmul s[q,k] (fp32, PE) evicted as fp16 via ACT.
 2. Per-row threshold by bisection on the fp16 values using
    tensor_scalar(is_ge)+accum_out row-counts (6 subsampled + 12 full rounds).
 3. Grid-align: round threshold up to the fp16 grid value g; exact fp32
    tie-resolution: recompute scores with a rank-1 shift (0.5 - g) baked into
    the matmul, extract the fp32 values of elements with fp16(s)==g via one
    scalar_tensor_tensor pass + vector.max (top-8), and pick the exact
    per-row threshold t so that count(s > t) == 205 (torch.quantile
    semantics: kept set == top-205, any t strictly inside the gap works).
 4. Layout-2 matmul s'[k,q] = K Q^T - t_q (rank-1 shift), ACT exp evict
    e' = exp(0.125 s') (fp16), DVE mask p' = (s' > 0) * e'.
 5. AV: [V|1] ^T-matmul accumulating over k -> att^T[d,q] + Z row; normalize
    by 1/Z (PE broadcast + DVE), block-diag distill matmul, gate mix,
    out-projection with bias folded in via ones-row.
"""

import os
import sys

sys.path.insert(0, "/opt/trn_rl_repo")

KSTAGE = int(os.environ.get("KSTAGE", "9"))
KSUB = int(os.environ.get("KSUB", "99"))

import numpy as np

import concourse.bass as bass
import concourse.mybir as mybir
from concourse import bacc
from concourse import bass_utils
from concourse.tile import TileContext
from contextlib import ExitStack

B, S, D, H = 4, 2048, 256, 4
HD = 64
NCORES = 8
NQT = S // 128      # 16 q (or k) tiles of 128
NKC = S // 512      # 4 free-dim chunks of 512

SUB_ROUNDS = 5
FULL_ROUNDS = 11
SUB_W = 512
WIDEN = 1.25
LOHI = 24.0

f32 = mybir.dt.float32
f16 = mybir.dt.float16
u16 = mybir.dt.uint16
u8 = mybir.dt.uint8
Alu = mybir.AluOpType
Act = mybir.ActivationFunctionType


def _build():
    nc = bacc.Bacc("TRN2", target_bir_lowering=False, debug=False,
                   num_devices=NCORES)

    # ---- DRAM I/O (identical program on all cores; per-core data) ----
    xT_d = nc.dram_tensor("xT", [D, S], f32, kind="ExternalInput")
    wqT_d = nc.dram_tensor("wqT", [D, 128], f32, kind="ExternalInput")
    wkT_d = nc.dram_tensor("wkT", [D, 128], f32, kind="ExternalInput")
    wvT_d = nc.dram_tensor("wvT", [D, 128], f32, kind="ExternalInput")
    bq_d = nc.dram_tensor("bq", [128, 1], f32, kind="ExternalInput")
    bk_d = nc.dram_tensor("bk", [128, 1], f32, kind="ExternalInput")
    bvr_d = nc.dram_tensor("bvr", [1, 128], f32, kind="ExternalInput")
    wdT_d = nc.dram_tensor("wdT", [2, 65, 64], f16, kind="ExternalInput")
    wgT_d = nc.dram_tensor("wgT", [D, 16], f32, kind="ExternalInput")
    bg_d = nc.dram_tensor("bg", [16, 1], f32, kind="ExternalInput")
    wgpT_d = nc.dram_tensor("wgpT", [16, 128], f32, kind="ExternalInput")
    bgp_d = nc.dram_tensor("bgp", [128, 1], f32, kind="ExternalInput")
    woTa_d = nc.dram_tensor("woTa", [128, 256], f16, kind="ExternalInput")

    outT_d = nc.dram_tensor("outT", [D, S], f32, kind="ExternalOutput")
    tdbg_d = nc.dram_tensor("tdbg", [128, 32], f32, kind="ExternalOutput")
    cdbg_d = nc.dram_tensor("cdbg", [128, 32], f32, kind="ExternalOutput")

    with TileContext(nc) as tc, ExitStack() as ctx:
        cst = ctx.enter_context(tc.tile_pool(name="cst", bufs=1))
        big = ctx.enter_context(tc.tile_pool(name="big", bufs=2))
        ebp = ctx.enter_context(tc.tile_pool(name="ebp", bufs=2))
        rot = ctx.enter_context(tc.tile_pool(name="rot", bufs=2))
        pmm = ctx.enter_context(tc.tile_pool(name="pmm", bufs=3, space="PSUM"))
        pav = ctx.enter_context(tc.tile_pool(name="pav", bufs=2, space="PSUM"))
        psm = ctx.enter_context(tc.tile_pool(name="psm", bufs=1, space="PSUM"))

        # ---- load constants ----
        xT = [ebp.tile([128, S], f32, tag="eb", name=f"xT{i}") for i in range(2)]
        nc.sync.dma_start(out=xT[0][:], in_=xT_d[0:128, :])
        nc.sync.dma_start(out=xT[1][:], in_=xT_d[128:256, :])
        wqT = [cst.tile([128, 128], f32, tag=f"wqT{i}", name=f"wqT{i}") for i in range(2)]
        wkT = [cst.tile([128, 128], f32, tag=f"wkT{i}", name=f"wkT{i}") for i in range(2)]
        wvT = [cst.tile([128, 128], f32, tag=f"wvT{i}", name=f"wvT{i}") for i in range(2)]
        for i in range(2):
            nc.sync.dma_start(out=wqT[i][:], in_=wqT_d[128 * i:128 * i + 128, :])
            nc.sync.dma_start(out=wkT[i][:], in_=wkT_d[128 * i:128 * i + 128, :])
            nc.sync.dma_start(out=wvT[i][:], in_=wvT_d[128 * i:128 * i + 128, :])
        bq = cst.tile([128, 1], f32, tag="bq", name="bq")
        bk = cst.tile([128, 1], f32, tag="bk", name="bk")
        bvr = cst.tile([1, 128], f32, tag="bvr", name="bvr")
        nc.sync.dma_start(out=bq[:], in_=bq_d[:])
        nc.sync.dma_start(out=bk[:], in_=bk_d[:])
        nc.sync.dma_start(out=bvr[:], in_=bvr_d[:])
        wdT = [cst.tile([65, 64], f16, tag=f"wdT{i}", name=f"wdT{i}") for i in range(2)]
        nc.sync.dma_start(out=wdT[0][:], in_=wdT_d[0])
        nc.sync.dma_start(out=wdT[1][:], in_=wdT_d[1])
        wgT = [cst.tile([128, 16], f32, tag=f"wgT{i}", name=f"wgT{i}") for i in range(2)]
        nc.sync.dma_start(out=wgT[0][:], in_=wgT_d[0:128, :])
        nc.sync.dma_start(out=wgT[1][:], in_=wgT_d[128:256, :])
        bg = cst.tile([16, 1], f32, tag="bg", name="bg")
        wgpT = cst.tile([16, 128], f32, tag="wgpT", name="wgpT")
        bgp = cst.tile([128, 1], f32, tag="bgp", name="bgp")
        nc.sync.dma_start(out=bg[:], in_=bg_d[:])
        nc.sync.dma_start(out=wgpT[:], in_=wgpT_d[:])
        nc.sync.dma_start(out=bgp[:], in_=bgp_d[:])
        woTa0 = cst.tile([64, 256], f16, tag="woTa0", name="woTa0")
        woTa1 = cst.tile([64, 256], f16, tag="woTa1", name="woTa1")
        nc.sync.dma_start(out=woTa0[:], in_=woTa_d[0:64, :])
        nc.sync.dma_start(out=woTa1[:], in_=woTa_d[64:128, :])

        rowsA = cst.tile([128, S], f32, tag="rowsA", name="rowsA")
        rowsB = cst.tile([128, S], f32, tag="rowsB", name="rowsB")
        ones = rowsA[0:1, :]
        ones32 = rowsA[32:33, :]
        ones64 = rowsA[64:65, :]
        tierow = rowsB[0:1, :]
        trow = rowsB[32:33, :]
        rz = rowsB[64:65, :]
        nc.vector.memset(ones, 1.0)
        nc.vector.memset(ones32, 1.0)
        nc.vector.memset(ones64, 1.0)
        iota8 = cst.tile([128, 8], f32, tag="iota8", name="iota8")
        for j in range(8):
            nc.vector.memset(iota8[:, j:j + 1], float(j))

        # ---- gate path: g = sigmoid(mean(x) @ Wg^T + bg); gd = Wgp g + bgp
        xm = [cst.tile([128, 1], f32, tag=f"xm{i}", name=f"xm{i}") for i in range(2)]
        nc.vector.reduce_sum(xm[0][:], xT[0][:], axis=mybir.AxisListType.X)
        nc.vector.reduce_sum(xm[1][:], xT[1][:], axis=mybir.AxisListType.X)
        psg = psm.tile([16, 1], f32, tag="ps_small", name="ps_small")
        nc.tensor.matmul(psg[:], wgT[0][:], xm[0][:], start=True, stop=False)
        nc.tensor.matmul(psg[:], wgT[1][:], xm[1][:], start=False, stop=True)
        gsig = cst.tile([16, 1], f32, tag="gsig", name="gsig")
        nc.scalar.activation(gsig[:], psg[:], Act.Sigmoid, bias=bg[:],
                             scale=1.0 / S)
        psgd = psm.tile([128, 1], f32, tag="ps_small", name="ps_small")
        nc.tensor.matmul(psgd[:], wgpT[:], gsig[:], start=True, stop=True)
        gd = cst.tile([128, 1], f32, tag="gd", name="gd")
        nc.scalar.activation(gd[:], psgd[:], Act.Identity, bias=bgp[:],
                             scale=1.0)

        # ---- QKV projections ----
        QT = cst.tile([128, S], f32, tag="QT", name="QT")
        KT = cst.tile([128, S], f32, tag="KT", name="KT")
        for nq in range(NKC):
            sl = slice(nq * 512, nq * 512 + 512)
            ps = pmm.tile([128, 512], f32, tag="ps_mm", name="ps_mm")
            nc.tensor.matmul(ps[:], wqT[0][:], xT[0][:, sl], start=True, stop=False)
            nc.tensor.matmul(ps[:], wqT[1][:], xT[1][:, sl], start=False, stop=True)
            nc.scalar.activation(QT[:, sl], ps[:], Act.Identity, bias=bq[:], scale=1.0)
            ps = pmm.tile([128, 512], f32, tag="ps_mm", name="ps_mm")
            nc.tensor.matmul(ps[:], wkT[0][:], xT[0][:, sl], start=True, stop=False)
            nc.tensor.matmul(ps[:], wkT[1][:], xT[1][:, sl], start=False, stop=True)
            nc.scalar.activation(KT[:, sl], ps[:], Act.Identity, bias=bk[:], scale=1.0)

        # V16 per k-tile layout: [V_h0 (64) | ones | V_h1 (64)] -> 129 cols
        VW = 130
        V16 = cst.tile([128, NQT * VW], f16, tag="V16", name="V16")
        for m in range(NQT):
            msl = slice(m * 128, m * 128 + 128)
            ps = pmm.tile([128, 128], f32, tag="ps_mm", name="ps_mm")
            nc.tensor.matmul(ps[:], xT[0][:, msl], wvT[0][:], start=True, stop=False)
            nc.tensor.matmul(ps[:], xT[1][:, msl], wvT[1][:], start=False, stop=False)
            nc.tensor.matmul(ps[:], ones[:, 0:128], bvr[:], start=False, stop=True)
            o = m * VW
            nc.scalar.activation(V16[:, o:o + 64], ps[:, 0:64], Act.Copy)
            nc.scalar.activation(V16[:, o + 65:o + 129], ps[:, 64:128], Act.Copy)
            nc.vector.memset(V16[:, o + 64:o + 65], 1.0)
            nc.vector.memset(V16[:, o + 129:o + 130], 1.0)

        # ---- per-head state tiles ----
        loA = cst.tile([128, NQT], f32, tag="loA", name="loA")
        loB = cst.tile([128, NQT], f32, tag="loB", name="loB")
        hiA = cst.tile([128, NQT], f32, tag="hiA", name="hiA")
        hiB = cst.tile([128, NQT], f32, tag="hiB", name="hiB")
        mid = cst.tile([128, NQT], f32, tag="mid", name="mid")
        cnt = cst.tile([128, NQT], f32, tag="cnt", name="cnt")
        sel = cst.tile([128, NQT], u8, tag="sel", name="sel")
        selu = cst.tile([128, NQT], u16, tag="selu", name="selu")
        g16 = cst.tile([128, NQT], f16, tag="g16", name="g16")
        gp16 = cst.tile([128, NQT], f16, tag="gp16", name="gp16")
        gf = cst.tile([128, NQT], f32, tag="gf", name="gf")
        gpf = cst.tile([128, NQT], f32, tag="gpf", name="gpf")
        tglow = cst.tile([128, NQT], f32, tag="tglow", name="tglow")
        cge = cst.tile([128, NQT], f32, tag="cge", name="cge")
        nT = cst.tile([128, NQT], f32, tag="nT", name="nT")
        jj = cst.tile([128, NQT], f32, tag="jj", name="jj")
        jj1 = cst.tile([128, NQT], f32, tag="jj1", name="jj1")
        tauj = cst.tile([128, NQT], f32, tag="tauj", name="tauj")
        tauj1 = cst.tile([128, NQT], f32, tag="tauj1", name="tauj1")
        tmid = cst.tile([128, NQT], f32, tag="tmid", name="tmid")
        tshift = cst.tile([128, NQT], f32, tag="tshift", name="tshift")
        tneg = cst.tile([128, NQT], f32, tag="tneg", name="tneg")
        shiftc = cst.tile([128, NQT], f32, tag="shiftc", name="shiftc")
        eb8 = cst.tile([128, NQT * 8], f32, tag="eb8", name="eb8")
        oh = cst.tile([128, 8], f32, tag="oh", name="oh")
        junk8 = cst.tile([128, 8], f32, tag="junk8", name="junk8")

        attn16 = cst.tile([128, S], f16, tag="attn16", name="attn16")
        mixT0 = cst.tile([64, S], f16, tag="mixT0", name="mixT0")
        mixT1 = cst.tile([64, S], f16, tag="mixT1", name="mixT1")
        gdh1 = cst.tile([64, 1], f32, tag="gdh1", name="gdh1")
        sc2 = cst.tile([128, 512], f32, tag="sc2", name="sc2")

        def rowgather(dst_row, src_col):
            # dst_row[0, qi*128 + p] = src_col[p, qi]
            for qi in range(NQT):
                nc.sync.dma_start(
                    out=dst_row[:, qi * 128:(qi + 1) * 128],
                    in_=src_col[:, qi:qi + 1])

        def stA(h):
            # layout-1 scores -> fp16 (PE -> ACT evict)
            qs = slice(64 * h, 64 * h + 64)
            scr = ebp.tile([128, S], f16, tag="eb", name=f"scr{h}")
            s16 = big.tile([128, NQT * S], f16, tag="big", name=f"s16_{h}")
            hs[h]["scr"], hs[h]["s16"] = scr, s16
            for qi in range(NQT):
                qsl = slice(qi * 128, qi * 128 + 128)
                for kc in range(NKC):
                    ksl = slice(kc * 512, kc * 512 + 512)
                    ps = pmm.tile([128, 512], f32, tag="ps_mm", name="ps_mm")
                    nc.tensor.matmul(ps[:], QT[qs, qsl], KT[qs, ksl],
                                     start=True, stop=True)
                    nc.scalar.activation(
                        s16[:, qi * S + kc * 512: qi * S + kc * 512 + 512],
                        ps[:], Act.Copy)

        def stB(h):
            # bisection on fp16 values (DVE) -> gf, tglow, cge
            scr, s16 = hs[h]["scr"], hs[h]["s16"]
            nc.vector.memset(loA[:], 0.0)
            nc.vector.memset(hiA[:], 12.0)
            cur_lo, cur_hi, alt_lo, alt_hi = loA, hiA, loB, hiB
            for r in range(SUB_ROUNDS + FULL_ROUNDS):
                sub = r < SUB_ROUNDS
                w = SUB_W if sub else S
                nc.vector.tensor_add(mid[:], cur_lo[:], cur_hi[:])
                nc.vector.tensor_scalar_mul(mid[:], mid[:], 0.5)
                for qi in range(NQT):
                    nc.vector.tensor_scalar(
                        out=scr[:, 0:w], in0=s16[:, qi * S: qi * S + w],
                        scalar1=mid[:, qi:qi + 1], scalar2=0.0,
                        op0=Alu.is_ge, op1=Alu.add,
                        accum_out=cnt[:, qi:qi + 1])
                tgt = 204.5 * (w / float(S))
                nc.vector.tensor_scalar(out=sel[:], in0=cnt[:], scalar1=tgt,
                                        scalar2=None, op0=Alu.is_ge)
                nc.vector.select(alt_lo[:], sel[:], mid[:], cur_lo[:])
                nc.vector.select(alt_hi[:], sel[:], cur_hi[:], mid[:])
                cur_lo, alt_lo = alt_lo, cur_lo
                cur_hi, alt_hi = alt_hi, cur_hi
                if r == SUB_ROUNDS - 1:
                    nc.vector.tensor_scalar_sub(cur_lo[:], cur_lo[:], WIDEN)
                    nc.vector.tensor_scalar_add(cur_hi[:], cur_hi[:], WIDEN)
            # grid-align lo to the fp16 grid; t_glow = boundary below grid g
            nc.vector.tensor_copy(g16[:], cur_lo[:])            # RNE cast
            nc.vector.tensor_copy(gf[:], g16[:])
            nc.vector.tensor_tensor(out=selu[:], in0=gf[:], in1=cur_lo[:],
                                    op=Alu.is_lt)
            nc.vector.tensor_tensor(out=g16[:].bitcast(u16),
                                    in0=g16[:].bitcast(u16), in1=selu[:],
                                    op=Alu.add)                 # round up
            nc.vector.tensor_scalar(out=gp16[:].bitcast(u16),
                                    in0=g16[:].bitcast(u16), scalar1=1,
                                    scalar2=None, op0=Alu.subtract)
            nc.vector.tensor_copy(gf[:], g16[:])
            nc.vector.tensor_copy(gpf[:], gp16[:])
            nc.vector.tensor_add(tglow[:], gf[:], gpf[:])
            nc.vector.tensor_scalar_mul(tglow[:], tglow[:], 0.5)
            for qi in range(NQT):
                nc.vector.tensor_scalar(
                    out=scr[:, 0:S], in0=s16[:, qi * S: qi * S + S],
                    scalar1=tglow[:, qi:qi + 1], scalar2=0.0,
                    op0=Alu.is_ge, op1=Alu.add, accum_out=cge[:, qi:qi + 1])

        def stC(h):
            # exact fp32 tie values via rank-1-shifted recompute + max8;
            # pick exact threshold t_shift, build -t row for layout-2
            qs = slice(64 * h, 64 * h + 64)
            s16 = hs[h]["s16"]
            nc.vector.tensor_scalar(out=shiftc[:], in0=gf[:], scalar1=-1.0,
                                    scalar2=0.5, op0=Alu.mult, op1=Alu.add)
            rowgather(tierow, shiftc)
            for qi in range(NQT):
                qsl = slice(qi * 128, qi * 128 + 128)
                eb = ebp.tile([128, S], f32, tag="eb", name="eb")
                for kc in range(NKC):
                    ksl = slice(kc * 512, kc * 512 + 512)
                    ps = pmm.tile([128, 512], f32, tag="ps_mm", name="ps_mm")
                    nc.tensor.matmul(ps[:], QT[qs, qsl], KT[qs, ksl],
                                     start=True, stop=False)
                    nc.tensor.matmul(ps[:], tierow[:, qsl],
                                     ones[:, ksl], start=False, stop=True)
                    nc.vector.scalar_tensor_tensor(
                        out=eb[:, ksl], in0=s16[:, qi * S + kc * 512:
                                                qi * S + kc * 512 + 512],
                        scalar=gf[:, qi:qi + 1], in1=ps[:],
                        op0=Alu.is_equal, op1=Alu.mult)
                nc.vector.max(out=eb8[:, qi * 8: qi * 8 + 8], in_=eb[:])
            # nT = count(eb8 > 0.25); j = 204 - (cge - nT); pick tau_j, tau_j1
            for qi in range(NQT):
                nc.vector.tensor_scalar(
                    out=junk8[:], in0=eb8[:, qi * 8: qi * 8 + 8],
                    scalar1=0.25, scalar2=0.0, op0=Alu.is_gt, op1=Alu.add,
                    accum_out=nT[:, qi:qi + 1])
            nc.vector.tensor_tensor(out=jj[:], in0=cge[:], in1=nT[:],
                                    op=Alu.subtract)        # c_hi
            nc.vector.tensor_scalar(out=jj[:], in0=jj[:], scalar1=-1.0,
                                    scalar2=204.0, op0=Alu.mult, op1=Alu.add)
            nc.vector.tensor_scalar_add(jj1[:], jj[:], 1.0)
            for qi in range(NQT):
                bsl = slice(qi * 8, qi * 8 + 8)
                nc.vector.tensor_scalar(out=oh[:], in0=iota8[:],
                                        scalar1=jj[:, qi:qi + 1],
                                        scalar2=None, op0=Alu.is_equal)
                nc.vector.tensor_tensor(out=junk8[:], in0=oh[:],
                                        in1=eb8[:, bsl], op=Alu.mult)
                nc.vector.reduce_sum(tauj[:, qi:qi + 1], junk8[:],
                                     axis=mybir.AxisListType.X)
                nc.vector.tensor_scalar(out=oh[:], in0=iota8[:],
                                        scalar1=jj1[:, qi:qi + 1],
                                        scalar2=None, op0=Alu.is_equal)
                nc.vector.tensor_tensor(out=junk8[:], in0=oh[:],
                                        in1=eb8[:, bsl], op=Alu.mult)
                nc.vector.reduce_sum(tauj1[:, qi:qi + 1], junk8[:],
                                     axis=mybir.AxisListType.X)
            # t_mid = gf - 0.5 + 0.5*(tauj + tauj1); keep-all rows use t_glow
            nc.vector.tensor_add(tmid[:], tauj[:], tauj1[:])
            nc.vector.tensor_scalar_mul(tmid[:], tmid[:], 0.5)
            nc.vector.tensor_add(tmid[:], tmid[:], gf[:])
            nc.vector.tensor_scalar_sub(tmid[:], tmid[:], 0.5)
            nc.vector.tensor_scalar(out=sel[:], in0=cge[:], scalar1=205.25,
                                    scalar2=None, op0=Alu.is_le)
            nc.vector.select(tshift[:], sel[:], tglow[:], tmid[:])
            nc.vector.tensor_scalar_mul(tneg[:], tshift[:], -1.0)
            nc.sync.dma_start(out=tdbg_d[:, h * 16:h * 16 + 16], in_=tshift[:])
            nc.sync.dma_start(out=cdbg_d[:, h * 16:h * 16 + 16], in_=cge[:])
            rowgather(trow, tneg)

        def stD(h):
            # layout-2: s' = K Q^T - t (rank-1); exp evict; mask p'
            qs = slice(64 * h, 64 * h + 64)
            pT = big.tile([128, NQT * S], f16, tag="big", name=f"pT{h}")
            hs[h]["pT"] = pT
            for ki in range(NQT):
                ksl = slice(ki * 128, ki * 128 + 128)
                for qc in range(NKC):
                    qsl2 = slice(qc * 512, qc * 512 + 512)
                    ps = pmm.tile([128, 512], f32, tag="ps_mm", name="ps_mm")
                    nc.tensor.matmul(ps[:], KT[qs, ksl], QT[qs, qsl2],
                                     start=True, stop=False)
                    nc.tensor.matmul(ps[:], ones32[:, 0:128],
                                     trow[:, qsl2], start=False, stop=True)
                    ebuf = rot.tile([128, 512], f16, tag="ebuf", name="ebuf")
                    nc.scalar.activation(ebuf[:], ps[:], Act.Exp, scale=0.125)
                    nc.vector.scalar_tensor_tensor(
                        out=pT[:, ki * S + qc * 512: ki * S + qc * 512 + 512],
                        in0=ps[:], scalar=0.0, in1=ebuf[:],
                        op0=Alu.is_gt, op1=Alu.mult)

        def stE(h):
            # AV: att^T rows 0..63 + Z at 64; normalize into attn16
            pT = hs[h]["pT"]
            for nq in range(NKC):
                qsl2 = slice(nq * 512, nq * 512 + 512)
                pa = pav.tile([128, 512], f32, tag="ps_av", name="ps_av")
                for ki in range(NQT):
                    vsl = slice(ki * VW + 65 * h, ki * VW + 65 * h + 65)
                    nc.tensor.matmul(pa[0:65, 0:512],
                                     V16[:, vsl],
                                     pT[:, ki * S + nq * 512:
                                        ki * S + nq * 512 + 512],
                                     start=(ki == 0), stop=(ki == NQT - 1))
                attn_c = rot.tile([65, 512], f32, tag="attn_c", name="attn_c",
                                  bufs=1)
                nc.scalar.activation(attn_c[:, :], pa[0:65, 0:512], Act.Copy)
                nc.vector.reciprocal(rz[:, qsl2], attn_c[64:65, :])
                pb = pmm.tile([128, 512], f32, tag="ps_mm", name="ps_mm")
                nc.tensor.matmul(pb[0:64, 0:512], ones64[:, 0:64],
                                 rz[:, qsl2], start=True, stop=True)
                nc.vector.tensor_tensor(out=attn16[0:64, qsl2],
                                        in0=attn_c[0:64, :],
                                        in1=pb[0:64, 0:512], op=Alu.mult)

        def stF(h):
            # distill (block-diag) + sigmoid-gate mix -> mixT[h]
            mixT = mixT0 if h == 0 else mixT1
            if h == 1:
                nc.sync.dma_start(out=gdh1[:], in_=gd[64:128, 0:1])
            gdh = gd[0:64, 0:1] if h == 0 else gdh1[:]
            nc.vector.memset(attn16[64:65, :], 1.0)
            for nq in range(NKC):
                qsl2 = slice(nq * 512, nq * 512 + 512)
                pd = pmm.tile([128, 512], f32, tag="ps_mm", name="ps_mm")
                nc.tensor.matmul(pd[0:64, 0:512], wdT[0][:],
                                 attn16[0:65, qsl2], start=True, stop=True)
                nc.vector.tensor_tensor(out=sc2[0:64, 0:512],
                                        in0=pd[0:64, 0:512],
                                        in1=attn16[0:64, qsl2],
                                        op=Alu.subtract)
                nc.vector.scalar_tensor_tensor(
                    out=mixT[0:64, qsl2], in0=sc2[0:64, 0:512],
                    scalar=gdh, in1=attn16[0:64, qsl2],
                    op0=Alu.mult, op1=Alu.add)

        # lockstep emission: head-1 front-end overlaps head-0 back-end
        hs = {0: {}, 1: {}}
        stA(0)
        stA(1)
        stB(0)
        stC(0)
        stB(1)
        stD(0)
        stC(1)
        stE(0)
        stF(0)
        stD(1)
        stE(1)
        stF(1)

        # ---- out projection: outT[do, q] = Wo_slice^T-matmul + bias row
        for m in range(2):
            msl = slice(m * 128, m * 128 + 128)
            for nq in range(NKC):
                qsl2 = slice(nq * 512, nq * 512 + 512)
                ps = pmm.tile([128, 512], f32, tag="ps_mm", name="ps_mm")
                nc.tensor.matmul(ps[:], woTa0[:, msl], mixT0[:, qsl2],
                                 start=True, stop=False)
                nc.tensor.matmul(ps[:], woTa1[:, msl], mixT1[:, qsl2],
                                 start=False, stop=True)
                oev = rot.tile([128, 512], f32, tag="oev", name="oev",
                               bufs=1)
                nc.scalar.activation(oev[:], ps[:], Act.Copy)
                nc.sync.dma_start(out=outT_d[msl, qsl2], in_=oev[:])

    nc.compile()
    return nc


def _host_prep(inputs):
    x = np.asarray(inputs["x"], np.float32)
    Wq = np.asarray(inputs["Wq"], np.float32); bq = np.asarray(inputs["bq"], np.float32)
    Wk = np.asarray(inputs["Wk"], np.float32); bk = np.asarray(inputs["bk"], np.float32)
    Wv = np.asarray(inputs["Wv"], np.float32); bv = np.asarray(inputs["bv"], np.float32)
    Wd = np.asarray(inputs["Wd"], np.float32); bd = np.asarray(inputs["bd"], np.float32)
    Wg = np.asarray(inputs["Wg"], np.float32); bg = np.asarray(inputs["bg"], np.float32)
    Wgp = np.asarray(inputs["Wgp"], np.float32); bgp = np.asarray(inputs["bgp"], np.float32)
    Wo = np.asarray(inputs["Wo"], np.float32); bo = np.asarray(inputs["bo"], np.float32)

    blk = np.zeros((64, 64), np.float32)
    for gg in range(4):
        blk[gg * 16:(gg + 1) * 16, gg * 16:(gg + 1) * 16] = Wd.T
    bdrep = np.tile(bd, 4).astype(np.float32)

    in_maps = []
    for c in range(NCORES):
        b, hp = c // 2, c % 2
        dsl = slice(128 * hp, 128 * hp + 128)
        wdT = np.zeros((2, 65, 64), np.float16)
        wdT[0] = np.vstack([blk, bdrep[None, :]]).astype(np.float16)
        wdT[1] = np.vstack([bdrep[None, :], blk]).astype(np.float16)
        woTa = np.zeros((128, 256), np.float16)
        for m in range(2):
            woTa[:, m * 128:(m + 1) * 128] = Wo[m * 128:(m + 1) * 128, dsl].T.astype(np.float16)
        in_maps.append(dict(
            xT=np.ascontiguousarray(x[b].T),
            wqT=np.ascontiguousarray(Wq[dsl].T),
            wkT=np.ascontiguousarray(Wk[dsl].T),
            wvT=np.ascontiguousarray(Wv[dsl].T),
            bq=bq[dsl].reshape(128, 1).copy(),
            bk=bk[dsl].reshape(128, 1).copy(),
            bvr=bv[dsl].reshape(1, 128).copy(),
            wdT=wdT,
            wgT=np.ascontiguousarray(Wg.T),
            bg=bg.reshape(16, 1).copy(),
            wgpT=np.ascontiguousarray(Wgp[dsl].T),
            bgp=bgp[dsl].reshape(128, 1).copy(),
            woTa=woTa,
        ))
    return in_maps


_prog_cache = {}


def kernel(**inputs) -> np.ndarray:
    if "nc" not in _prog_cache:
        _prog_cache["nc"] = _build()
    nc = _prog_cache["nc"]
    in_maps = _host_prep(inputs)
    res = bass_utils.run_bass_kernel_spmd(nc, in_maps,
                                          core_ids=list(range(NCORES)))
    out = np.zeros((B, S, D), np.float32)
    bo = np.asarray(inputs["bo"], np.float32)
    for b in range(B):
        acc = res.results[2 * b]["outT"] + res.results[2 * b + 1]["outT"]
        out[b] = acc.T + bo
    return out


if __name__ == "__main__":
    print("use test.py")



# revision 18
# speedup vs baseline: 2.5435x; 2.5435x over previous
"""Trainium2 Bass kernel for nn_DynamicSparseAttention (v2).

Model (per batch b): QKV proj -> per-head scores [S,S] -> per-row exact 0.9
quantile threshold -> sparse mask (top ~205 of 2048 per row) -> softmax ->
att @ V -> block-diag distill (16x16) -> sigmoid gate mix -> out proj.

Sharding: 8 cores = (batch b in 0..3) x (head-pair hp in 0..1); each core
computes 2 heads of one batch and a partial out-projection [256, 2048]^T.
Host sums the two partials per batch.

v2 strategy (vs v1 fp16-grid bisection):
 - Scores computed in FP32 (PE 4 cyc/row) and kept in SBUF as fp32; the
   per-row threshold is found by bracketed regula falsi (2 Gaussian-model
   warm rounds + 6 secant rounds) running directly on the fp32 values.
   Counting passes are split DVE (tensor_scalar is_ge accum) / ACT (Sign
   activation accum); bracket bookkeeping arithmetic on GPSIMD, selects on
   DVE. Rows that hit count==205 freeze their tau; the rest use the last
   interpolated tau.
 - stD recomputes scores layout-2 with the threshold merged as a 65th
   contraction row (rank-1 shift), masks with is_ge so the kept set is
   exactly the counted set, exp evict fp16, AV accumulate, distill+gate,
   fp16 out-projection.
 - Unit pipeline: 4 units (head, q-half) flow through stA -> bisect ->
   stD/stE/stF with bisect(u+1) interleaved against stD(u) so DVE/ACT/PE
   all stay busy.
"""

import os
import sys

sys.path.insert(0, "/opt/trn_rl_repo")

import numpy as np

import concourse.bass as bass
import concourse.mybir as mybir
from concourse import bacc
from concourse import bass_utils
from concourse.tile import TileContext
from contextlib import ExitStack

B, S, D, H = 4, 2048, 256, 4
HD = 64
NCORES = 8
NKC = S // 512          # 4 free-dim chunks of 512
NQT_U = 8               # q-tiles (128 rows) per unit
UQ = NQT_U * 128        # 1024 q rows per unit

ROUNDS = int(os.environ.get("RF_ROUNDS", "8"))
NGAUSS = int(os.environ.get("RF_NGAUSS", "2"))
DVE_QI = int(os.environ.get("RF_DVE_QI", "4"))  # of 8 count cols on DVE

f32 = mybir.dt.float32
f16 = mybir.dt.float16
i32 = mybir.dt.int32
u8 = mybir.dt.uint8
f8 = mybir.dt.float8e4
Alu = mybir.AluOpType
Act = mybir.ActivationFunctionType

QUAKE = 0x5F3759DF


def _build():
    nc = bacc.Bacc("TRN2", target_bir_lowering=False, debug=False,
                   num_devices=NCORES)

    xT_d = nc.dram_tensor("xT", [D, S], f32, kind="ExternalInput")
    wqT_d = nc.dram_tensor("wqT", [D, 128], f32, kind="ExternalInput")
    wkT_d = nc.dram_tensor("wkT", [D, 128], f32, kind="ExternalInput")
    wvT_d = nc.dram_tensor("wvT", [D, 128], f32, kind="ExternalInput")
    bq_d = nc.dram_tensor("bq", [128, 1], f32, kind="ExternalInput")
    bk_d = nc.dram_tensor("bk", [128, 1], f32, kind="ExternalInput")
    bvr_d = nc.dram_tensor("bvr", [1, 128], f32, kind="ExternalInput")
    wdT_d = nc.dram_tensor("wdT", [2, 65, 64], f16, kind="ExternalInput")
    wgT_d = nc.dram_tensor("wgT", [D, 16], f32, kind="ExternalInput")
    bgh_d = nc.dram_tensor("bgh", [16, 1], f32, kind="ExternalInput")
    wgpT_d = nc.dram_tensor("wgpT", [16, 128], f32, kind="ExternalInput")
    bgp_d = nc.dram_tensor("bgp", [128, 1], f32, kind="ExternalInput")
    woTa_d = nc.dram_tensor("woTa", [128, 256], f16, kind="ExternalInput")

    outT_d = nc.dram_tensor("outT", [D, S], f16, kind="ExternalOutput")
    tdbg_d = nc.dram_tensor("tdbg", [128, 32], f32, kind="ExternalOutput")
    cdbg_d = nc.dram_tensor("cdbg", [128, 64], f32, kind="ExternalOutput")
    bdbg_d = nc.dram_tensor("bdbg", [128, 64], f32, kind="ExternalOutput")
    adbg_d = nc.dram_tensor("adbg", [128, 32], u8, kind="ExternalOutput")

    with TileContext(nc) as tc, ExitStack() as ctx:
        cst = ctx.enter_context(tc.tile_pool(name="cst", bufs=1))
        big = ctx.enter_context(tc.tile_pool(name="big", bufs=2))
        ptp = ctx.enter_context(tc.tile_pool(name="ptp", bufs=1))
        rot = ctx.enter_context(tc.tile_pool(name="rot", bufs=2))
        st2 = ctx.enter_context(tc.tile_pool(name="st2", bufs=2))
        pmm = ctx.enter_context(tc.tile_pool(name="pmm", bufs=3, space="PSUM"))
        pav = ctx.enter_context(tc.tile_pool(name="pav", bufs=2, space="PSUM"))
        pmx = ctx.enter_context(tc.tile_pool(name="pmx", bufs=2, space="PSUM"))
        psm = ctx.enter_context(tc.tile_pool(name="psm", bufs=1, space="PSUM"))

        # ---- load inputs ----
        xT = [big.tile([128, S], f32, tag="big64", name=f"xT{i}")
              for i in range(2)]
        nc.sync.dma_start(out=xT[0][:], in_=xT_d[0:128, :])
        nc.sync.dma_start(out=xT[1][:], in_=xT_d[128:256, :])
        wqT = [cst.tile([128, 128], f32, tag=f"wqT{i}", name=f"wqT{i}") for i in range(2)]
        wkT = [cst.tile([128, 128], f32, tag=f"wkT{i}", name=f"wkT{i}") for i in range(2)]
        wvT = [cst.tile([128, 128], f32, tag=f"wvT{i}", name=f"wvT{i}") for i in range(2)]
        for i in range(2):
            nc.sync.dma_start(out=wqT[i][:], in_=wqT_d[128 * i:128 * i + 128, :])
            nc.sync.dma_start(out=wkT[i][:], in_=wkT_d[128 * i:128 * i + 128, :])
            nc.sync.dma_start(out=wvT[i][:], in_=wvT_d[128 * i:128 * i + 128, :])
        bq = cst.tile([128, 1], f32, tag="bq", name="bq")
        bk = cst.tile([128, 1], f32, tag="bk", name="bk")
        bvr = cst.tile([1, 128], f32, tag="bvr", name="bvr")
        nc.sync.dma_start(out=bq[:], in_=bq_d[:])
        nc.sync.dma_start(out=bk[:], in_=bk_d[:])
        nc.sync.dma_start(out=bvr[:], in_=bvr_d[:])
        wdT = [cst.tile([65, 64], f16, tag=f"wdT{i}", name=f"wdT{i}") for i in range(2)]
        nc.sync.dma_start(out=wdT[0][:], in_=wdT_d[0])
        nc.sync.dma_start(out=wdT[1][:], in_=wdT_d[1])
        wgT = [cst.tile([128, 16], f32, tag=f"wgT{i}", name=f"wgT{i}") for i in range(2)]
        nc.sync.dma_start(out=wgT[0][:], in_=wgT_d[0:128, :])
        nc.sync.dma_start(out=wgT[1][:], in_=wgT_d[128:256, :])
        bgh = cst.tile([16, 1], f32, tag="bgh", name="bgh")
        wgpT = cst.tile([16, 128], f32, tag="wgpT", name="wgpT")
        bgp = cst.tile([128, 1], f32, tag="bgp", name="bgp")
        nc.sync.dma_start(out=bgh[:], in_=bgh_d[:])
        nc.sync.dma_start(out=wgpT[:], in_=wgpT_d[:])
        nc.sync.dma_start(out=bgp[:], in_=bgp_d[:])
        woTa0 = cst.tile([64, 256], f16, tag="woTa0", name="woTa0")
        woTa1 = cst.tile([64, 256], f16, tag="woTa1", name="woTa1")
        nc.sync.dma_start(out=woTa0[:], in_=woTa_d[0:64, :])
        nc.sync.dma_start(out=woTa1[:], in_=woTa_d[64:128, :])

        rowsA = cst.tile([65, 64], f16, tag="rowsA", name="rowsA")
        rowsB = cst.tile([65, 512], f16, tag="rowsB", name="rowsB")
        ones64h = rowsA[64:65, 0:64]
        rzrow = rowsB[64:65, 0:512]
        nc.vector.memset(ones64h, 1.0)
        onesf = cst.tile([1, 128], f32, tag="onesf", name="onesf")
        nc.vector.memset(onesf[:], 1.0)

        # ---- gate: g = sigmoid(mean(x) Wg^T + bg) via tanh; gd = Wgp g + bgp
        xm = [cst.tile([128, 1], f32, tag=f"xm{i}", name=f"xm{i}") for i in range(2)]
        nc.vector.reduce_sum(xm[0][:], xT[0][:], axis=mybir.AxisListType.X)
        nc.vector.reduce_sum(xm[1][:], xT[1][:], axis=mybir.AxisListType.X)
        psg = psm.tile([16, 1], f32, tag="ps_small", name="ps_small")
        nc.tensor.matmul(psg[:], wgT[0][:], xm[0][:], start=True, stop=False)
        nc.tensor.matmul(psg[:], wgT[1][:], xm[1][:], start=False, stop=True)
        gth = cst.tile([16, 1], f32, tag="gth", name="gth")
        # tanh(psg*0.5/S + bg/2) ; sigmoid(z) = 0.5 + 0.5*tanh(z/2)
        nc.scalar.activation(gth[:], psg[:], Act.Tanh, bias=bgh[:],
                             scale=0.5 / S)
        gsig = cst.tile([16, 1], f32, tag="gsig", name="gsig")
        nc.vector.tensor_scalar(out=gsig[:], in0=gth[:], scalar1=0.5,
                                scalar2=0.5, op0=Alu.mult, op1=Alu.add)
        psgd = psm.tile([128, 1], f32, tag="ps_small", name="ps_small")
        nc.tensor.matmul(psgd[:], wgpT[:], gsig[:], start=True, stop=True)
        gd = cst.tile([128, 1], f32, tag="gd", name="gd")
        nc.scalar.activation(gd[:], psgd[:], Act.Identity, bias=bgp[:],
                             scale=1.0)
        gdh1 = cst.tile([64, 1], f32, tag="gdh1", name="gdh1")
        nc.sync.dma_start(out=gdh1[:], in_=gd[64:128, 0:1])

        # ---- per-head Q/K in [65, S] layout: rows 0..63 head dims,
        #      row 64 = ones (K) / -tau (Q, filled after bisect) ----
        QTx = [cst.tile([128, S], f32, tag=f"QTx{h}", name=f"QTx{h}") for h in range(2)]
        KTx = [cst.tile([128, S], f32, tag=f"KTx{h}", name=f"KTx{h}") for h in range(2)]
        for h in range(2):
            hs64 = slice(64 * h, 64 * h + 64)
            for nq in range(NKC):
                sl = slice(nq * 512, nq * 512 + 512)
                ps = pmm.tile([128, 512], f32, tag="ps_mm", name="ps_mm")
                nc.tensor.matmul(ps[0:64, :], wqT[0][:, hs64], xT[0][:, sl],
                                 start=True, stop=False)
                nc.tensor.matmul(ps[0:64, :], wqT[1][:, hs64], xT[1][:, sl],
                                 start=False, stop=True)
                nc.scalar.activation(QTx[h][0:64, sl], ps[0:64, :],
                                     Act.Identity, bias=bq[hs64, 0:1], scale=1.0)
                ps = pmm.tile([128, 512], f32, tag="ps_mm", name="ps_mm")
                nc.tensor.matmul(ps[0:64, :], wkT[0][:, hs64], xT[0][:, sl],
                                 start=True, stop=False)
                nc.tensor.matmul(ps[0:64, :], wkT[1][:, hs64], xT[1][:, sl],
                                 start=False, stop=True)
                nc.scalar.activation(KTx[h][0:64, sl], ps[0:64, :],
                                     Act.Identity, bias=bk[hs64, 0:1], scale=1.0)
            nc.vector.memset(KTx[h][64:65, :], 1.0)

        # ---- V16 per k-tile layout: [V_h0 (64) | 1 | V_h1 (64) | 1] ----
        VW = 130
        NQT = S // 128
        V16 = cst.tile([128, NQT * VW], f16, tag="V16", name="V16")
        for m in range(NQT):
            msl = slice(m * 128, m * 128 + 128)
            ps = pmm.tile([128, 512], f32, tag="ps_mm", name="ps_mm")
            nc.tensor.matmul(ps[:, 0:128], xT[0][:, msl], wvT[0][:], start=True, stop=False)
            nc.tensor.matmul(ps[:, 0:128], xT[1][:, msl], wvT[1][:], start=False, stop=False)
            nc.tensor.matmul(ps[:, 0:128], onesf[:, 0:128], bvr[:], start=False, stop=True)
            o = m * VW
            nc.scalar.activation(V16[:, o:o + 64], ps[:, 0:64], Act.Identity)
            nc.scalar.activation(V16[:, o + 65:o + 129], ps[:, 64:128], Act.Identity)
            nc.vector.memset(V16[:, o + 64:o + 65], 1.0)
            nc.vector.memset(V16[:, o + 129:o + 130], 1.0)

        # ---- per-unit state ----
        # unit u: h = u // 2, half = u % 2, q in [half*1024, half*1024+1024)
        NU = 4

        attn16 = cst.tile([128, 512], f16, tag="attn16", name="attn16")
        nc.vector.memset(attn16[64:65, :], 1.0)
        sc2 = cst.tile([64, 512], f16, tag="sc2", name="sc2")
        mixT = [cst.tile([64, S], f16, tag=f"mixT{h}", name=f"mixT{h}")
                for h in range(2)]
        junkD = cst.tile([128, S], u8, tag="junkD", name="junkD")
        junkA = cst.tile([128, S], f8, tag="junkA", name="junkA")

        def new_state(u):
            t = {}
            for nm in ("mus", "s2", "mu", "var", "sd", "tau", "tauf",
                       "lo", "hi", "clo", "chi", "invd", "w1", "w2", "w3",
                       "tneg"):
                width = 32 if nm == "mus" else 8
                t[nm] = st2.tile([128, width], f32, tag=f"u{nm}",
                                 name=f"{nm}_{u}", bufs=2)
            t["iw"] = st2.tile([128, 8], i32, tag="uiw", name=f"iw_{u}", bufs=2)
            for nm in ("hit", "ge", "lt", "sel2", "any"):
                t[nm] = st2.tile([128, 8], u8, tag=f"u{nm}", name=f"{nm}_{u}",
                                 bufs=2)
            t["cnt"] = [st2.tile([128, 8], f32, tag=f"ucnt{i}",
                                 name=f"cnt{i}_{u}", bufs=2) for i in range(2)]
            return t

        def stA(u, sv):
            h, half = u // 2, u % 2
            s32 = big.tile([128, NQT_U * S], f32, tag="big64", name=f"s32_{u}")
            sv["s32"] = s32
            for qi in range(NQT_U):
                qsl = slice(half * UQ + qi * 128, half * UQ + qi * 128 + 128)
                for kc in range(NKC):
                    ksl = slice(kc * 512, kc * 512 + 512)
                    ps = pmm.tile([128, 512], f32, tag="ps_mm", name="ps_mm")
                    nc.tensor.matmul(ps[:], QTx[h][0:64, qsl],
                                     KTx[h][0:64, ksl], start=True, stop=True)
                    nc.scalar.activation(
                        s32[:, qi * S + kc * 512: qi * S + kc * 512 + 512],
                        ps[:], Act.Identity,
                        accum_out=sv["mus"][:, qi * 4 + kc: qi * 4 + kc + 1])

        def warm(u, sv):
            # mu (exact, from eviction accums); var from 512-sample E[s^2]
            nc.vector.reduce_sum(
                sv["mu"][:], sv["mus"][:].rearrange("p (a b) -> p a b", a=8),
                axis=mybir.AxisListType.X)
            nc.vector.tensor_scalar_mul(sv["mu"][:], sv["mu"][:], 1.0 / S)
            for qi in range(NQT_U):
                nc.scalar.activation(junkA[:, 0:512],
                                     sv["s32"][:, qi * S: qi * S + 512],
                                     Act.Square,
                                     accum_out=sv["s2"][:, qi:qi + 1])
            nc.vector.tensor_tensor(out=sv["w1"][:], in0=sv["mu"][:],
                                    in1=sv["mu"][:], op=Alu.mult)
            nc.vector.tensor_scalar(out=sv["var"][:], in0=sv["s2"][:],
                                    scalar1=1.0 / 512, scalar2=None,
                                    op0=Alu.mult)
            nc.vector.tensor_tensor(out=sv["var"][:], in0=sv["var"][:],
                                    in1=sv["w1"][:], op=Alu.subtract)
            nc.vector.tensor_scalar_max(sv["var"][:], sv["var"][:], 1e-6)
            # quake rsqrt + 2 Newton iterations -> w2 = 1/sqrt(var)
            nc.vector.tensor_scalar(out=sv["iw"][:],
                                    in0=sv["var"][:].bitcast(i32),
                                    scalar1=1, scalar2=None,
                                    op0=Alu.logical_shift_right)
            # QUAKE - x == (x ^ 0xffffffff) + (QUAKE + 1)
            nc.vector.tensor_scalar(out=sv["iw"][:], in0=sv["iw"][:],
                                    scalar1=-1, scalar2=None,
                                    op0=Alu.bitwise_xor)
            nc.vector.tensor_scalar(out=sv["iw"][:], in0=sv["iw"][:],
                                    scalar1=QUAKE + 1, scalar2=None,
                                    op0=Alu.add)
            r = sv["w2"]
            nc.vector.tensor_copy(r[:], sv["iw"][:].bitcast(f32))
            for _ in range(2):
                nc.vector.tensor_tensor(out=sv["w3"][:], in0=r[:], in1=r[:],
                                        op=Alu.mult)
                nc.vector.tensor_tensor(out=sv["w3"][:], in0=sv["w3"][:],
                                        in1=sv["var"][:], op=Alu.mult)
                nc.vector.tensor_scalar(out=sv["w3"][:], in0=sv["w3"][:],
                                        scalar1=-0.5, scalar2=1.5,
                                        op0=Alu.mult, op1=Alu.add)
                nc.vector.tensor_tensor(out=r[:], in0=r[:], in1=sv["w3"][:],
                                        op=Alu.mult)
            nc.vector.tensor_tensor(out=sv["sd"][:], in0=sv["var"][:],
                                    in1=r[:], op=Alu.mult)
            # tau0 = mu + 1.2816 sd ; invd0 = sd / 359.2
            nc.vector.scalar_tensor_tensor(out=sv["tau"][:], in0=sv["sd"][:],
                                           scalar=1.2816, in1=sv["mu"][:],
                                           op0=Alu.mult, op1=Alu.add)
            nc.vector.tensor_scalar(out=sv["invd"][:], in0=sv["sd"][:],
                                    scalar1=1.0 / 359.2, scalar2=None,
                                    op0=Alu.mult)
            # bracket init
            nc.vector.memset(sv["lo"][:], -30.0)
            nc.vector.memset(sv["hi"][:], 30.0)
            nc.vector.memset(sv["clo"][:], float(S))
            nc.vector.memset(sv["chi"][:], 0.0)
            nc.vector.memset(sv["tauf"][:], 0.0)
            nc.vector.memset(sv["any"][:], 0)

        def bis_round(u, sv, r):
            s32, cnt = sv["s32"], sv["cnt"][r % 2]
            tau = sv["tau"]
            for qi in range(NQT_U):
                ssl = slice(qi * S, qi * S + S)
                if qi < DVE_QI:
                    nc.vector.tensor_scalar(
                        out=junkD[:], in0=s32[:, ssl],
                        scalar1=tau[:, qi:qi + 1], scalar2=0.0,
                        op0=Alu.is_ge, op1=Alu.add,
                        accum_out=cnt[:, qi:qi + 1])
                else:
                    nc.scalar.activation(
                        junkA[:], s32[:, ssl], Act.Sign,
                        bias=tau[:, qi:qi + 1], scale=-1.0,
                        accum_out=cnt[:, qi:qi + 1])

        def bis_book(u, sv, r):
            cnt = sv["cnt"][r % 2]
            tau, tauf = sv["tau"], sv["tauf"]
            lo, hi, clo, chi = sv["lo"], sv["hi"], sv["clo"], sv["chi"]
            acols = slice(DVE_QI, 8)
            # ACT sign-sum -> count: c = 1024 - 0.5*acc
            nc.vector.tensor_scalar(out=cnt[:, acols], in0=cnt[:, acols],
                                    scalar1=-0.5, scalar2=1024.0,
                                    op0=Alu.mult, op1=Alu.add)
            nc.vector.tensor_scalar(out=sv["hit"][:], in0=cnt[:],
                                    scalar1=205.0, scalar2=None,
                                    op0=Alu.is_equal)
            nc.vector.select(tauf[:], sv["hit"][:], tau[:], tauf[:])
            nc.vector.tensor_tensor(out=sv["any"][:], in0=sv["any"][:],
                                    in1=sv["hit"][:], op=Alu.max)
            nc.vector.tensor_scalar(out=sv["ge"][:], in0=cnt[:],
                                    scalar1=204.5, scalar2=None,
                                    op0=Alu.is_ge)
            ge = sv["ge"]
            lt = sv["lt"]
            nc.vector.tensor_scalar(out=lt[:], in0=cnt[:], scalar1=204.5,
                                    scalar2=None, op0=Alu.is_lt)
            nc.vector.select(lo[:], ge[:], tau[:], lo[:])
            nc.vector.select(clo[:], ge[:], cnt[:], clo[:])
            nc.vector.select(hi[:], lt[:], tau[:], hi[:])
            nc.vector.select(chi[:], lt[:], cnt[:], chi[:])
            if r < NGAUSS:
                # tau += (c - 204.5) * invd0 ; op0(in0, scalar) = cnt - 204.5
                nc.vector.scalar_tensor_tensor(
                    out=sv["w1"][:], in0=cnt[:], scalar=204.5,
                    in1=sv["invd"][:], op0=Alu.subtract, op1=Alu.mult)
                nc.vector.tensor_tensor(out=tau[:], in0=tau[:],
                                        in1=sv["w1"][:], op=Alu.add)
            else:
                nc.vector.tensor_tensor(out=sv["w1"][:], in0=hi[:], in1=lo[:],
                                        op=Alu.subtract)          # width
                nc.vector.tensor_tensor(out=sv["w2"][:], in0=clo[:],
                                        in1=chi[:], op=Alu.subtract)  # dc
                nc.vector.reciprocal(sv["w2"][:], sv["w2"][:])
                nc.vector.tensor_tensor(out=sv["invd"][:], in0=sv["w1"][:],
                                        in1=sv["w2"][:], op=Alu.mult)
                nc.vector.tensor_tensor(out=sv["w3"][:], in0=clo[:],
                                        in1=chi[:], op=Alu.add)
                nc.vector.tensor_scalar(out=sv["sel2"][:], in0=sv["w3"][:],
                                        scalar1=410.0, scalar2=None,
                                        op0=Alu.is_le)
                nc.vector.select(sv["w1"][:], sv["sel2"][:], lo[:], hi[:])
                nc.vector.select(sv["w2"][:], sv["sel2"][:], clo[:], chi[:])
                nc.vector.tensor_scalar(out=sv["w2"][:], in0=sv["w2"][:],
                                        scalar1=204.5, scalar2=None,
                                        op0=Alu.subtract)
                nc.vector.tensor_tensor(out=sv["w2"][:], in0=sv["w2"][:],
                                        in1=sv["invd"][:], op=Alu.mult)
                nc.vector.tensor_tensor(out=tau[:], in0=sv["w1"][:],
                                        in1=sv["w2"][:], op=Alu.add)
                nc.vector.tensor_tensor(out=tau[:], in0=tau[:], in1=lo[:],
                                        op=Alu.max)
                nc.vector.tensor_tensor(out=tau[:], in0=tau[:], in1=hi[:],
                                        op=Alu.min)

        def post_bisect(u, sv):
            h, half = u // 2, u % 2
            nc.vector.select(sv["tau"][:], sv["any"][:], sv["tauf"][:],
                             sv["tau"][:])
            nc.vector.tensor_scalar_mul(sv["tneg"][:], sv["tau"][:], -1.0)
            nc.sync.dma_start(out=tdbg_d[:, u * 8:u * 8 + 8], in_=sv["tau"][:])
            nc.sync.dma_start(out=cdbg_d[:, u * 16:u * 16 + 8], in_=sv["clo"][:])
            nc.sync.dma_start(out=cdbg_d[:, u * 16 + 8:u * 16 + 16], in_=sv["chi"][:])
            nc.sync.dma_start(out=bdbg_d[:, u * 16:u * 16 + 8], in_=sv["lo"][:])
            nc.sync.dma_start(out=bdbg_d[:, u * 16 + 8:u * 16 + 16], in_=sv["hi"][:])
            nc.sync.dma_start(out=adbg_d[:, u * 8:u * 8 + 8], in_=sv["any"][:])
            for qi in range(NQT_U):
                qs = half * UQ + qi * 128
                nc.sync.dma_start(out=QTx[h][64:65, qs:qs + 128],
                                  in_=sv["tneg"][:, qi:qi + 1])

        def stD_slice(u, sv, idx):
            # 32 (ki,qc) score tiles per unit; emit 4 per call (idx 0..7)
            h, half = u // 2, u % 2
            for j in range(4):
                t = idx * 4 + j
                qc, ki = t // 16, t % 16
                ksl = slice(ki * 128, ki * 128 + 128)
                qsl = slice(half * UQ + qc * 512, half * UQ + qc * 512 + 512)
                ps = pmm.tile([128, 512], f32, tag="ps_mm", name="ps_mm")
                nc.tensor.matmul(ps[:], KTx[h][0:65, ksl], QTx[h][0:65, qsl],
                                 start=True, stop=True)
                ebuf = rot.tile([128, 512], f16, tag="ebuf", name="ebuf")
                nc.scalar.activation(ebuf[:], ps[:], Act.Exp, scale=0.125)
                pT = sv["pT"][qc]
                nc.vector.scalar_tensor_tensor(
                    out=pT[:, ki * 512: ki * 512 + 512],
                    in0=ps[:], scalar=0.0, in1=ebuf[:],
                    op0=Alu.is_ge, op1=Alu.mult)

        def stEF(u, sv, qc):
            h, half = u // 2, u % 2
            pT = sv["pT"][qc]
            qsl = slice(half * UQ + qc * 512, half * UQ + qc * 512 + 512)
            pa = pav.tile([128, 512], f32, tag="ps_av", name="ps_av")
            for ki in range(NQT):
                vsl = slice(ki * VW + 65 * h, ki * VW + 65 * h + 65)
                nc.tensor.matmul(pa[0:65, :], V16[:, vsl],
                                 pT[:, ki * 512: ki * 512 + 512],
                                 start=(ki == 0), stop=(ki == NQT - 1))
            attn_c = rot.tile([65, 512], f16, tag="attn_c", name="attn_c",
                              bufs=1)
            nc.scalar.activation(attn_c[:, :], pa[0:65, :], Act.Identity)
            # Z reciprocal via transpose-gather: [1,512] -> [128,4]
            zt = rot.tile([128, 4], f16, tag="zt", name="zt", bufs=1)
            for j in range(4):
                nc.sync.dma_start(out=zt[:, j:j + 1],
                                  in_=attn_c[64:65, j * 128:j * 128 + 128])
            rz4h = rot.tile([128, 4], f16, tag="rz4h", name="rz4h", bufs=1)
            with nc.allow_low_precision(reason="1/Z in f16 is plenty for softmax normalize"):
                nc.vector.reciprocal(rz4h[:], zt[:])
            for j in range(4):
                nc.sync.dma_start(out=rzrow[:, j * 128:j * 128 + 128],
                                  in_=rz4h[:, j:j + 1])
            pb = pmx.tile([64, 512], f32, tag="ps_mx", name="ps_mx")
            nc.tensor.matmul(pb[:], ones64h[:, 0:64], rzrow[:, 0:512],
                             start=True, stop=True)
            nc.vector.tensor_tensor(out=attn16[0:64, :], in0=attn_c[0:64, :],
                                    in1=pb[:], op=Alu.mult)
            # distill + gate mix
            gdh = gd[0:64, 0:1] if h == 0 else gdh1[:]
            pd = pmx.tile([64, 512], f32, tag="ps_mx", name="ps_mx")
            nc.tensor.matmul(pd[:], wdT[h][:], attn16[0:65, :],
                             start=True, stop=True)
            nc.vector.tensor_tensor(out=sc2[:], in0=pd[:],
                                    in1=attn16[0:64, :], op=Alu.subtract)
            nc.vector.scalar_tensor_tensor(
                out=mixT[h][:, qsl], in0=sc2[:], scalar=gdh,
                in1=attn16[0:64, :], op0=Alu.mult, op1=Alu.add)

        def outproj(qc_glob):
            qsl = slice(qc_glob * 512, qc_glob * 512 + 512)
            for m in range(2):
                msl = slice(m * 128, m * 128 + 128)
                ps = pmm.tile([128, 512], f32, tag="ps_mm", name="ps_mm")
                nc.tensor.matmul(ps[:], woTa0[:, msl], mixT[0][:, qsl],
                                 start=True, stop=False)
                nc.tensor.matmul(ps[:], woTa1[:, msl], mixT[1][:, qsl],
                                 start=False, stop=True)
                oev = rot.tile([128, 512], f16, tag="oev", name="oev",
                               bufs=1)
                nc.scalar.activation(oev[:], ps[:], Act.Identity)
                nc.sync.dma_start(out=outT_d[msl, qsl], in_=oev[:])

        # ---- pipeline ----
        svs = [new_state(u) for u in range(4)]
        for u in range(4):
            svs[u]["pT"] = None

        def make_pT(sv):
            t = ptp.tile([128, NQT * 512], f16, tag="pTc", name="pTc")
            sv["pT"] = [t, t]

        stA(0, svs[0])
        warm(0, svs[0])
        stA(1, svs[1])
        warm(1, svs[1])
        # bisect(0) alone (interleaves naturally with stA(1) evictions)
        for r in range(ROUNDS):
            bis_round(0, svs[0], r)
            bis_book(0, svs[0], r)
        post_bisect(0, svs[0])

        def run_unit(u_bis, u_d):
            # interleave bisect(u_bis) rounds with stD/stE(u_d)
            make_pT(svs[u_d])
            for r in range(ROUNDS):
                if u_bis is not None:
                    bis_round(u_bis, svs[u_bis], r)
                stD_slice(u_d, svs[u_d], r)
                if u_bis is not None:
                    bis_book(u_bis, svs[u_bis], r)
                if r == 3:
                    stEF(u_d, svs[u_d], 0)
            if u_bis is not None:
                post_bisect(u_bis, svs[u_bis])
            stEF(u_d, svs[u_d], 1)

        run_unit(1, 0)
        stA(2, svs[2])
        warm(2, svs[2])
        run_unit(2, 1)
        stA(3, svs[3])
        warm(3, svs[3])
        run_unit(3, 2)
        run_unit(None, 3)
        for qc_glob in range(NKC):
            outproj(qc_glob)

    nc.compile()
    return nc


def _host_prep(inputs):
    x = np.asarray(inputs["x"], np.float32)
    Wq = np.asarray(inputs["Wq"], np.float32); bq = np.asarray(inputs["bq"], np.float32)
    Wk = np.asarray(inputs["Wk"], np.float32); bk = np.asarray(inputs["bk"], np.float32)
    Wv = np.asarray(inputs["Wv"], np.float32); bv = np.asarray(inputs["bv"], np.float32)
    Wd = np.asarray(inputs["Wd"], np.float32); bd = np.asarray(inputs["bd"], np.float32)
    Wg = np.asarray(inputs["Wg"], np.float32); bg = np.asarray(inputs["bg"], np.float32)
    Wgp = np.asarray(inputs["Wgp"], np.float32); bgp = np.asarray(inputs["bgp"], np.float32)
    Wo = np.asarray(inputs["Wo"], np.float32); bo = np.asarray(inputs["bo"], np.float32)

    blk = np.zeros((64, 64), np.float32)
    for gg in range(4):
        blk[gg * 16:(gg + 1) * 16, gg * 16:(gg + 1) * 16] = Wd.T
    bdrep = np.tile(bd, 4).astype(np.float32)

    in_maps = []
    for c in range(NCORES):
        b, hp = c // 2, c % 2
        dsl = slice(128 * hp, 128 * hp + 128)
        wdT = np.zeros((2, 65, 64), np.float16)
        wdT[0] = np.vstack([blk, bdrep[None, :]]).astype(np.float16)
        wdT[1] = np.vstack([blk, bdrep[None, :]]).astype(np.float16)
        woTa = np.zeros((128, 256), np.float16)
        for m in range(2):
            woTa[:, m * 128:(m + 1) * 128] = Wo[m * 128:(m + 1) * 128, dsl].T.astype(np.float16)
        in_maps.append(dict(
            xT=np.ascontiguousarray(x[b].T),
            wqT=np.ascontiguousarray(Wq[dsl].T),
            wkT=np.ascontiguousarray(Wk[dsl].T),
            wvT=np.ascontiguousarray(Wv[dsl].T),
            bq=bq[dsl].reshape(128, 1).copy(),
            bk=bk[dsl].reshape(128, 1).copy(),
            bvr=bv[dsl].reshape(1, 128).copy(),
            wdT=wdT,
            wgT=np.ascontiguousarray(Wg.T),
            bgh=(bg * 0.5).reshape(16, 1).copy(),
            wgpT=np.ascontiguousarray(Wgp[dsl].T),
            bgp=bgp[dsl].reshape(128, 1).copy(),
            woTa=woTa,
        ))
    return in_maps


_prog_cache = {}


def kernel(**inputs) -> np.ndarray:
    if "nc" not in _prog_cache:
        _prog_cache["nc"] = _build()
    nc = _prog_cache["nc"]
    in_maps = _host_prep(inputs)
    res = bass_utils.run_bass_kernel_spmd(nc, in_maps,
                                          core_ids=list(range(NCORES)))
    out = np.zeros((B, S, D), np.float32)
    bo = np.asarray(inputs["bo"], np.float32)
    for b in range(B):
        acc = (res.results[2 * b]["outT"].astype(np.float32)
               + res.results[2 * b + 1]["outT"].astype(np.float32))
        out[b] = acc.T + bo
    return out


if __name__ == "__main__":
    print("use test.py")
